# revision 1
# baseline (speedup 1.0000x reference)
"""ASPP pooling head on Trainium2 (Bass/Tile), data-parallel over batch on 8 cores.

Computation per sample:
    pooled = mean(x, spatial)            # [Cin]
    y      = relu((pooled @ W.T) * bn_scale + bn_shift)   # [Cout]
    out    = broadcast(y, spatial)       # [Cout, H, W]

Device kernel per core (2 samples), winning config (measured floor ~206 us,
~= 77.6 MB of mandatory HBM traffic at the ~420 GB/s per-core streaming rate
plus fixed NEFF overhead; f32 end-to-end, rel err ~1e-6):
    - x streamed as 32x [128ch, 4096] f32 tiles (2 MiB DMAs, 9-deep pool)
      on SyncE HWDGE; fine granularity + deep queue rides out HBM-stack
      contention from the paired core; wt/shift const loads are deferred
      into the stream (after the first x tiles) so the ramp starts on x
    - spatial sums on VectorE tensor_reduce (free-dim, 1x mode, ~4.4 us/tile)
    - 16 accumulating PE matmuls per (sample, o-block):
      psum[128o, 1] += wt_chunk[128c, 128o].T @ pooled[128c, 1],
      with wt = (W * bn_scale / 4096).T folded on host
    - broadcast = one pass over the output bytes: Relu(psum_bcast + shift)
      via ScalarE activation with a stride-0 source AP; stores via ScalarE
      HWDGE (second ring, no head-of-line blocking with input stream)
"""

import numpy as np

B, CIN, H, W_SP = 16, 2048, 64, 64
COUT = 256
NCORES = 8
BPC = B // NCORES          # samples per core
SP = H * W_SP              # 4096 spatial positions
KCH = CIN // 128           # 16 channel chunks of 128
CPT = 2                    # channel chunks per x tile (4 MiB DMA)
NOB = COUT // 128          # output-channel blocks
BN_EPS = 1e-5

_CACHE = {}

VARIANTS = {
    "v1": dict(split_reduce=False, bcast_halves=1, bcast_dve_ob1=False,
               consts_gpsimd=False, xin_bufs=4),
    "v3": dict(split_reduce=True, bcast_halves=2, bcast_dve_ob1=True,
               consts_gpsimd=True, xin_bufs=4),
    "v4": dict(split_reduce=True, bcast_halves=2, bcast_dve_ob1=True,
               consts_gpsimd=True, xin_bufs=4, alt_in_queue=True),
    "v5": dict(split_reduce=False, bcast_halves=2, bcast_dve_ob1=False,
               consts_gpsimd=True, xin_bufs=4),
    "v5d": dict(split_reduce=False, bcast_halves=2, bcast_dve_ob1=True,
                consts_gpsimd=True, xin_bufs=4),
    "v1g": dict(split_reduce=False, bcast_halves=1, bcast_dve_ob1=False,
                consts_gpsimd=True, xin_bufs=4),
    "v1c1": dict(split_reduce=False, bcast_halves=1, bcast_dve_ob1=False,
                 consts_gpsimd=False, xin_bufs=8, cpt=1),
    "v1c4": dict(split_reduce=False, bcast_halves=1, bcast_dve_ob1=False,
                 consts_gpsimd=False, xin_bufs=2, cpt=4),
    "v1c1b": dict(split_reduce=False, bcast_halves=1, bcast_dve_ob1=False,
                  consts_gpsimd=False, xin_bufs=9, cpt=1),
    "v1h1": dict(split_reduce=False, bcast_halves=1, bcast_dve_ob1=False,
                 consts_gpsimd=False, xin_bufs=8, cpt=1, dma_splits=2),
    "v1c1c": dict(split_reduce=False, bcast_halves=2, bcast_dve_ob1=False,
                  consts_gpsimd=False, xin_bufs=10, cpt=1),
    "v1c1q": dict(split_reduce=False, bcast_halves=2, bcast_dve_ob1=False,
                  consts_gpsimd=False, xin_bufs=10, cpt=1, alt_in_queue=True),
    "v1c1s6": dict(split_reduce=False, bcast_halves=1, bcast_dve_ob1=False,
                   consts_gpsimd=False, xin_bufs=6, cpt=1),
    "v6": dict(split_reduce=False, bcast_halves=1, bcast_dve_ob1=False,
               consts_gpsimd=False, xin_bufs=8, cpt=1, alt_reduce=True),
    "v7": dict(split_reduce=False, bcast_halves=1, bcast_dve_ob1=True,
               consts_gpsimd=False, xin_bufs=8, cpt=1, alt_reduce=True,
               consts_late=True),
    "v8": dict(split_reduce=False, bcast_halves=1, bcast_dve_ob1=False,
               consts_gpsimd=False, xin_bufs=9, cpt=1, alt_in_queue=True),
    "v9": dict(split_reduce=False, bcast_halves=1, bcast_dve_ob1=False,
               consts_gpsimd=False, xin_bufs=18, half_tiles=True),
    "v10": dict(split_reduce=False, bcast_halves=1, bcast_dve_ob1=False,
                consts_gpsimd=False, xin_bufs=9, cpt=1, consts_late=True),
}


def _build_nc(split_reduce=False, bcast_halves=1, bcast_dve_ob1=False,
              consts_gpsimd=False, xin_bufs=9, cpt=1,
              alt_in_queue=False, dma_splits=1, alt_reduce=False,
              consts_late=True, half_tiles=False):
    import concourse.bacc as bacc
    import concourse.mybir as mybir
    import concourse.tile as tile

    nc = bacc.Bacc("TRN2", target_bir_lowering=False, debug=False,
                   num_devices=NCORES)
    f32 = mybir.dt.float32
    AT = mybir.ActivationFunctionType
    x = nc.dram_tensor("x", [BPC, CIN, SP], f32, kind="ExternalInput").ap()
    wt = nc.dram_tensor("wt", [CIN, COUT], f32, kind="ExternalInput").ap()
    shift = nc.dram_tensor("shift", [COUT], f32, kind="ExternalInput").ap()
    out = nc.dram_tensor("out", [BPC, COUT, SP], f32, kind="ExternalOutput").ap()

    hsp = SP // bcast_halves
    cdma = nc.gpsimd.dma_start if consts_gpsimd else nc.sync.dma_start

    with tile.TileContext(nc) as tc, \
         tc.tile_pool(name="consts", bufs=1) as consts, \
         tc.tile_pool(name="xin", bufs=xin_bufs) as xin, \
         tc.tile_pool(name="pooled", bufs=8) as pooledp, \
         tc.tile_pool(name="psum", bufs=2, space="PSUM") as psump, \
         tc.tile_pool(name="bcast", bufs=2) as bcastp:

        # wt laid out [128 (c within chunk), KCH, COUT]; chunk k's o-block ob
        # is columns k*COUT + ob*128 ...  Emission may be deferred into the
        # first tiles' DMA stream (consts_late) to keep the ramp on x.
        wt_sb = consts.tile([128, KCH * COUT], f32)
        shift_sb = consts.tile([128, NOB], f32)
        wt_r = wt.rearrange("(k p) o -> p k o", p=128)
        wt_d = wt_sb[:].rearrange("p (k o) -> p k o", k=KCH)
        hk = KCH // 2

        def emit_consts(step):
            if not consts_late and step == 0:
                cdma(wt_d, wt_r)
                cdma(shift_sb[:], shift.rearrange("(ob p) -> p ob", p=128))
            elif consts_late and step == 1:
                cdma(wt_d[:, :hk], wt_r[:, :hk])
            elif consts_late and step == 2:
                cdma(wt_d[:, hk:], wt_r[:, hk:])
            elif consts_late and step == 3:
                cdma(shift_sb[:], shift.rearrange("(ob p) -> p ob", p=128))

        emit_consts(0)
        if split_reduce or alt_reduce:
            zeros_col = consts.tile([128, 1], f32)
            nc.gpsimd.memset(zeros_col[:], 0.0)
            scratch = consts.tile([128, SP], f32)

        for b in range(BPC):
            pss = [psump.tile([128, 1], f32, name=f"ps{ob}", tag=f"ps{ob}")
                   for ob in range(NOB)]
            if half_tiles:
                # 1 MiB tiles: each (chunk k, spatial half h) is its own
                # reduce + accumulating matmul step (same wt chunk for both
                # halves; PSUM does the pairwise sum).
                for k in range(KCH):
                    for hh in range(2):
                        xt = xin.tile([128, SP // 2], f32, name="xt",
                                      tag="xt")
                        nc.sync.dma_start(
                            xt[:], x[b, k * 128:(k + 1) * 128,
                                     hh * (SP // 2):(hh + 1) * (SP // 2)])
                        pt = pooledp.tile([128, 1], f32, name="pt", tag="pt")
                        nc.vector.reduce_sum(pt[:], xt[:],
                                             axis=mybir.AxisListType.X)
                        for ob in range(NOB):
                            nc.tensor.matmul(
                                pss[ob][:],
                                lhsT=wt_sb[:, k * COUT + ob * 128:
                                           k * COUT + ob * 128 + 128],
                                rhs=pt[:, 0:1],
                                start=(k == 0 and hh == 0),
                                stop=(k == KCH - 1 and hh == 1),
                            )
                    if b == 0 and k < 3:
                        emit_consts(k + 1)
            for kt in range(0 if half_tiles else KCH // cpt):
                xt = xin.tile([128, cpt, SP], f32)
                src = x[b, kt * cpt * 128:(kt + 1) * cpt * 128, :] \
                    .rearrange("(c p) s -> p c s", p=128)
                in_eng = nc.scalar if (alt_in_queue and kt % 2) else nc.sync
                if dma_splits == 1:
                    in_eng.dma_start(xt[:], src)
                else:
                    dsp = SP // dma_splits
                    for dd in range(dma_splits):
                        in_eng.dma_start(xt[:, :, dd * dsp:(dd + 1) * dsp],
                                         src[:, :, dd * dsp:(dd + 1) * dsp])
                if b == 0 and kt < 3:
                    emit_consts(kt + 1)
                if alt_reduce and kt % 2 == 1:
                    pta = pooledp.tile([128, 1], f32, name="pta", tag="pta")
                    nc.scalar.activation(scratch[:], xt[:, 0, :], AT.Identity,
                                         bias=zeros_col[:], scale=1.0,
                                         accum_out=pta[:])
                    parts = ((0, pta),)
                elif split_reduce:
                    ptv = pooledp.tile([128, 1], f32, name="ptv", tag="ptv")
                    nc.vector.reduce_sum(ptv[:], xt[:, 0, :],
                                         axis=mybir.AxisListType.X)
                    pta = pooledp.tile([128, 1], f32, name="pta", tag="pta")
                    nc.scalar.activation(scratch[:], xt[:, 1, :], AT.Identity,
                                         bias=zeros_col[:], scale=1.0,
                                         accum_out=pta[:])
                    parts = ((0, ptv), (1, pta))
                else:
                    pt = pooledp.tile([128, cpt], f32, name="pt", tag="pt")
                    nc.vector.reduce_sum(pt[:], xt[:],
                                         axis=mybir.AxisListType.X)
                    parts = tuple((c, pt[:, c:c + 1]) for c in range(cpt))
                for c, pcol in parts:
                    k = kt * cpt + c
                    for ob in range(NOB):
                        nc.tensor.matmul(
                            pss[ob][:],
                            lhsT=wt_sb[:, k * COUT + ob * 128:
                                       k * COUT + ob * 128 + 128],
                            rhs=pcol[:, 0:1],
                            start=(k == 0),
                            stop=(k == KCH - 1),
                        )
            for ob in range(NOB):
                for h in range(bcast_halves):
                    bc = bcastp.tile([128, hsp], f32, name=f"bc{ob}",
                                     tag="bc")
                    src_b = pss[ob][:].broadcast_to([128, hsp])
                    if ob == 1 and bcast_dve_ob1:
                        nc.vector.tensor_scalar(
                            out=bc[:], in0=src_b,
                            scalar1=shift_sb[:, ob:ob + 1], scalar2=0.0,
                            op0=mybir.AluOpType.add, op1=mybir.AluOpType.max)
                    else:
                        nc.scalar.activation(bc[:], src_b, AT.Relu,
                                             bias=shift_sb[:, ob:ob + 1],
                                             scale=1.0)
                    nc.scalar.dma_start(
                        out[b, ob * 128:(ob + 1) * 128,
                            h * hsp:(h + 1) * hsp], bc[:])

    nc.compile()
    return nc


def _prep_inputs(x, W, gamma, beta, running_mean, running_var):
    scale = np.asarray(gamma, np.float32) / np.sqrt(
        np.asarray(running_var, np.float32) + np.float32(BN_EPS))
    wt = np.ascontiguousarray(
        (np.asarray(W, np.float32) * scale[:, None]).T / np.float32(SP))
    shift = (np.asarray(beta, np.float32)
             - np.asarray(running_mean, np.float32) * scale).astype(np.float32)
    xs = np.ascontiguousarray(np.asarray(x, np.float32)).reshape(
        NCORES, BPC, CIN, SP)
    return [{"x": xs[i], "wt": wt, "shift": shift} for i in range(NCORES)]


def kernel(x, W, gamma, beta, running_mean, running_var):
    from concourse import bass_utils

    if "nc" not in _CACHE:
        _CACHE["nc"] = _build_nc()
    nc = _CACHE["nc"]
    in_maps = _prep_inputs(x, W, gamma, beta, running_mean, running_var)
    res = bass_utils.run_bass_kernel_spmd(nc, in_maps,
                                          core_ids=list(range(NCORES)))
    outs = [res.results[i]["out"] for i in range(NCORES)]
    return np.concatenate(outs, axis=0).reshape(B, COUT, H, W_SP)



# revision 2
# speedup vs baseline: 1.0636x; 1.0636x over previous
"""ASPP pooling head on Trainium2 (Bass/Tile), data-parallel over batch on 8 cores.

Per sample: pooled = mean(x, spatial); y = relu((pooled @ W.T)*bn_scale + bn_shift);
out = broadcast(y, spatial).

Per core (2 samples): 64 MiB of x reads + output writes + 2 MiB weights, all
bounded by the ~436 GB/s per-core DMA/HBM ceiling (measured; matches the
16-engine SDMA aggregate).  Design, from trace analysis:
  - x streamed as 32x [128ch, 4096] f32 tiles (2 MiB) on the sync HWDGE queue
    (the fastest path; SWDGE/gpsimd casts measured ~10% slower).
  - wt is host-prearranged to [128, KCH*COUT] so its load is one contiguous
    DMA on the scalar queue (the naive rearranged AP generated 2048x1KB
    descriptors and 32us of HWDGE sequencer burn, starving the matmuls).
  - spatial sums: VectorE tensor_reduce, with a subset of chunks offloaded to
    ScalarE (activation accum_out) so the reduce chain never lags the stream
    even when the chip clocks down ~20% under load.
  - 16 accumulating PE matmuls per (sample, o-block) with wt = W*bn_scale/4096
    folded on host.
  - broadcast: Relu(psum_bcast + shift) via ScalarE activation (stride-0 src)
    and DVE tensor_scalar for the last sample's second block; output stored as
    bf16 (halves write traffic and the tail; host upcasts; rel err ~3e-3 vs
    the 2e-2 gate).
  - tail: the last two chunks stream as half-tiles with alternating
    ScalarE/DVE reduces so the final reduce completes ~2us after the last
    byte lands; last-sample stores split across both HWDGE queues.
"""

import numpy as np

B, CIN, H, W_SP = 16, 2048, 64, 64
COUT = 256
NCORES = 8
BPC = B // NCORES
SP = H * W_SP
KCH = CIN // 128
NOB = COUT // 128
BN_EPS = 1e-5

# winning config (g2)
SCA_CHUNKS = (1, 4, 7, 10, 13)  # chunks reduced on ScalarE
TAIL_CHUNKS = 2               # last-sample chunks streamed as half-tiles
XIN_BUFS = 8
POOLED_BUFS = 24
HALVES = 2                    # bcast/store pieces per o-block
OUT_BF16 = True
DUAL_ALL = False              # DVE bcast for ob1 only on the last sample

_CACHE = {}


def _build_nc():
    import concourse.bacc as bacc
    import concourse.mybir as mybir
    import concourse.tile as tile

    nc = bacc.Bacc("TRN2", target_bir_lowering=False, debug=False,
                   num_devices=NCORES)
    f32 = mybir.dt.float32
    odt = mybir.dt.bfloat16 if OUT_BF16 else f32
    AT = mybir.ActivationFunctionType
    x = nc.dram_tensor("x", [BPC, CIN, SP], f32, kind="ExternalInput").ap()
    wt = nc.dram_tensor("wt", [128, KCH * COUT], f32, kind="ExternalInput").ap()
    shift = nc.dram_tensor("shift", [COUT], f32, kind="ExternalInput").ap()
    out = nc.dram_tensor("out", [BPC, COUT, SP], odt,
                         kind="ExternalOutput").ap()

    hsp = SP // HALVES

    with tile.TileContext(nc) as tc, \
         tc.tile_pool(name="consts", bufs=1) as consts, \
         tc.tile_pool(name="xin", bufs=XIN_BUFS) as xin, \
         tc.tile_pool(name="pooled", bufs=POOLED_BUFS) as pooledp, \
         tc.tile_pool(name="psum", bufs=2, space="PSUM") as psump, \
         tc.tile_pool(name="bcast", bufs=3) as bcastp:

        wt_sb = consts.tile([128, KCH * COUT], f32)
        shift_sb = consts.tile([128, NOB], f32)
        nc.scalar.dma_start(wt_sb[:], wt)
        nc.scalar.dma_start(shift_sb[:], shift.rearrange("(ob p) -> p ob", p=128))
        zeros_col = consts.tile([128, 1], f32)
        nc.gpsimd.memset(zeros_col[:], 0.0)
        scratch = consts.tile([128, SP], f32)

        for b in range(BPC):
            last = b == BPC - 1
            pss = [psump.tile([128, 1], f32, name=f"ps{ob}", tag=f"ps{ob}")
                   for ob in range(NOB)]

            def reduce_and_mm(src_slice, width, k, scalar_eng, first, stop):
                xt = xin.tile([128, width], f32, name="xt", tag="xt")
                nc.sync.dma_start(xt[:], src_slice)
                pt = pooledp.tile([128, 1], f32, name="pt", tag="pt")
                if scalar_eng:
                    nc.scalar.activation(scratch[:, :width], xt[:],
                                         AT.Identity, bias=zeros_col[:],
                                         scale=1.0, accum_out=pt[:])
                else:
                    nc.vector.reduce_sum(pt[:], xt[:],
                                         axis=mybir.AxisListType.X)
                for ob in range(NOB):
                    nc.tensor.matmul(
                        pss[ob][:],
                        lhsT=wt_sb[:, k * COUT + ob * 128:
                                   k * COUT + ob * 128 + 128],
                        rhs=pt[:, 0:1],
                        start=first,
                        stop=stop,
                    )

            for k in range(KCH):
                src = x[b, k * 128:(k + 1) * 128, :]
                tail_half = last and k >= KCH - TAIL_CHUNKS
                if (b == 0 and k == 0) or tail_half:
                    for d in range(2):
                        sca = tail_half and d == 0
                        reduce_and_mm(src[:, d * (SP // 2):(d + 1) * (SP // 2)],
                                      SP // 2, k, sca,
                                      first=(b == 0 and k == 0 and d == 0),
                                      stop=(k == KCH - 1 and d == 1))
                    continue
                reduce_and_mm(src, SP, k, k in SCA_CHUNKS,
                              first=(k == 0), stop=(k == KCH - 1))

            dual = DUAL_ALL or last
            for h in range(HALVES):
                for ob in range(NOB):
                    bc = bcastp.tile([128, hsp], odt, name=f"bc{ob}", tag="bc")
                    src_b = pss[ob][:].broadcast_to([128, hsp])
                    if dual and ob == 1:
                        nc.vector.tensor_scalar(
                            out=bc[:], in0=src_b,
                            scalar1=shift_sb[:, ob:ob + 1], scalar2=0.0,
                            op0=mybir.AluOpType.add, op1=mybir.AluOpType.max)
                    else:
                        nc.scalar.activation(bc[:], src_b, AT.Relu,
                                             bias=shift_sb[:, ob:ob + 1],
                                             scale=1.0)
                    st_eng = nc.sync if (last and ob == 1) else nc.scalar
                    st_eng.dma_start(
                        out[b, ob * 128:(ob + 1) * 128,
                            h * hsp:(h + 1) * hsp], bc[:])

    nc.compile()
    return nc


def _prep_inputs(x, W, gamma, beta, running_mean, running_var):
    scale = np.asarray(gamma, np.float32) / np.sqrt(
        np.asarray(running_var, np.float32) + np.float32(BN_EPS))
    wt = np.ascontiguousarray(
        (np.asarray(W, np.float32) * scale[:, None]).T / np.float32(SP))
    wt_r = np.ascontiguousarray(
        wt.reshape(KCH, 128, COUT).transpose(1, 0, 2).reshape(128, KCH * COUT))
    shift = (np.asarray(beta, np.float32)
             - np.asarray(running_mean, np.float32) * scale).astype(np.float32)
    xs = np.ascontiguousarray(np.asarray(x, np.float32)).reshape(
        NCORES, BPC, CIN, SP)
    return [{"x": xs[i], "wt": wt_r, "shift": shift} for i in range(NCORES)]


def kernel(x, W, gamma, beta, running_mean, running_var):
    from concourse import bass_utils

    if "nc" not in _CACHE:
        _CACHE["nc"] = _build_nc()
    nc = _CACHE["nc"]
    in_maps = _prep_inputs(x, W, gamma, beta, running_mean, running_var)
    res = bass_utils.run_bass_kernel_spmd(nc, in_maps,
                                          core_ids=list(range(NCORES)))
    outs = [np.asarray(res.results[i]["out"]).astype(np.float32)
            for i in range(NCORES)]
    return np.concatenate(outs, axis=0).reshape(B, COUT, H, W_SP)
